# revision 51
# baseline (speedup 1.0000x reference)
"""BitLinear forward kernel for Trainium2 (8 NeuronCores, data-parallel).

Computes y = sign(x) @ (alpha * code)^T + b where code/alpha are the
per-row ternarization of W (BitNet-style, delta_w = 0.05, delta_a = 0.0
so the activation quant is exactly sign(x)).

Sharding: x is split over batch*seq (16384 rows) across 8 cores; W is
replicated; outputs are concatenated on the host.

v2 design (no DRAM bounce, no DMA transpose):
- W row-tiles [128, 2048]: mean/thr/alpha stats via accumulate passes
  (Pool: -mean, ACT: |Wc| + T, Sign; DVE: s01 + den, num, code fp16).
- Transposes on the PE (matmul-with-identity) in 16-bit, landing in
  1-bank PSUM tiles of 8 d-subtiles; evicted with a dtype-converting
  pass: ACT Sign for x (raw bf16 -> fp8 in {-1,0,1}), DVE copy for code
  (fp16 -> fp8). Operand layout [128(d), 16(di), N] fp8; the DoubleRow
  pair dim is di pairs with stride 2048.
- Matmul in fp8 DoubleRow (8 passes of 256 contraction rows), fp32 PSUM;
  alpha applied on eviction (Pool) from a broadcast [128, O] tile.
- All pipelines interleaved per 1/16 step so DMA (48 MB/core) stays the
  bottleneck.
"""

import sys

for _p in ("/opt/trn_rl_repo", "/opt/trn_rl_repo/concourse"):
    if _p not in sys.path:
        sys.path.insert(0, _p)

import numpy as np

import concourse.bass as bass
import concourse.tile as tile
import concourse.mybir as mybir
from concourse import bacc
from concourse import masks
from concourse.bass_utils import run_bass_kernel_spmd

# Problem shape (hardcoded per contract)
B, S, D, O = 4, 4096, 2048, 2048
N_CORES = 8
T = (B * S) // N_CORES  # 2048 token rows per core
DELTA_W = 0.05

P = 128
TT = T // P   # 16 t-tiles
DT = D // P   # 16 d-tiles
WT = O // P   # 16 W row-tiles
DP = DT // 2  # 8 DoubleRow pairs
OB = 4        # o blocks (512 wide)
OBW = O // OB  # 512
HB = 8        # d-subtiles per PSUM transpose bank

F32 = mybir.dt.float32
BF16 = mybir.dt.bfloat16
FP16 = mybir.dt.float16
FP8 = mybir.dt.float8e4

_CACHE = {}


def _build(with_bias: bool):
    nc = bacc.Bacc("TRN2", target_bir_lowering=False, debug=False,
                   num_devices=N_CORES)
    x_d = nc.dram_tensor("x", [T, D], F32, kind="ExternalInput").ap()
    w_d = nc.dram_tensor("W", [O, D], F32, kind="ExternalInput").ap()
    y_d = nc.dram_tensor("y", [T, O], BF16, kind="ExternalOutput").ap()
    if with_bias:
        b_d = nc.dram_tensor("b", [O], F32, kind="ExternalInput").ap()

    with tile.TileContext(nc) as tc:
        with (
            tc.tile_pool(name="dram", bufs=2, space="DRAM") as dram,
            tc.tile_pool(name="wload", bufs=3) as wload,
            tc.tile_pool(name="xload", bufs=3) as xload,
            tc.tile_pool(name="wtmp", bufs=3) as wtmp,
            tc.tile_pool(name="junk", bufs=2) as junk_pool,
            tc.tile_pool(name="stats", bufs=1) as stats,
            tc.tile_pool(name="bigx", bufs=TT) as bigx,
            tc.tile_pool(name="bigc", bufs=OB) as bigc,
            tc.tile_pool(name="bcasta", bufs=OB) as bcasta,
            tc.tile_pool(name="yout", bufs=6) as yout,
            tc.tile_pool(name="bcast", bufs=1) as bcast,
            tc.tile_pool(name="idp", bufs=1) as idp,
            tc.tile_pool(name="ps_trx", bufs=2, space="PSUM") as ps_trx,
            tc.tile_pool(name="ps_trw", bufs=2, space="PSUM") as ps_trw,
            tc.tile_pool(name="ps_mm", bufs=4, space="PSUM") as ps_mm,
        ):
            ident_b = idp.tile([P, P], BF16, tag="ident_b")
            masks.make_identity(nc, ident_b[:])

            # matmul operands, split per consumer group to avoid false
            # dependencies: xqT per t-tile, codeT per o-block.
            # Layout: [128 (d within tile), di, N] fp8.
            xq_tiles = [bigx.tile([P, DT * P], FP8, tag="xqT",
                                  name=f"xqT{ti}") for ti in range(TT)]
            code_tiles = [bigc.tile([P, DT * OBW], FP8, tag="codeT",
                                    name=f"codeT{ob}") for ob in range(OB)]
            xqT_vs = [t[:].rearrange("p (di t) -> p di t", di=DT)
                      for t in xq_tiles]
            codeT_vs = [t[:].rearrange("p (di o) -> p di o", di=DT)
                        for t in code_tiles]

            alphaB_tiles = [bcasta.tile([P, OBW], F32, tag="alphaB",
                                        name=f"alphaB{ob}") for ob in range(OB)]
            alpha_dram = dram.tile([O], F32, tag="alphad")

            # per-o-row stats, one column per W row-tile
            negmean_all = stats.tile([P, WT], F32, tag="negmean")
            T_all = stats.tile([P, WT], F32, tag="T")
            thr_all = stats.tile([P, WT], F32, tag="thr")
            negthr_all = stats.tile([P, WT], F32, tag="negthr")
            den_all = stats.tile([P, WT], F32, tag="den")
            R_all = stats.tile([P, WT], F32, tag="R")
            num_all = stats.tile([P, WT], F32, tag="num")
            denc_all = stats.tile([P, WT], F32, tag="denc")
            rden_all = stats.tile([P, WT], F32, tag="rden")
            alpha_all = stats.tile([P, WT], F32, tag="alpha")

            if with_bias:
                biasB = bcast.tile([P, O], F32, tag="biasB")
                nc.sync.dma_start(
                    biasB[:], b_d.unsqueeze(0).to_broadcast((P, O)))

            xbufs = {}
            wbufs = {}

            def x_load(ti):
                xb = xload.tile([P, D], BF16)
                nc.gpsimd.dma_start(xb[:], x_d[ti * P:(ti + 1) * P, :])
                xbufs[ti] = xb

            def x_stage(ti):
                xb = xbufs.pop(ti)
                for h in range(DT // HB):
                    pst = ps_trx.tile([P, HB * P], BF16, tag="pstx")
                    for j in range(HB):
                        di = h * HB + j
                        nc.tensor.transpose(
                            pst[:, j * P:(j + 1) * P],
                            xb[:, di * P:(di + 1) * P],
                            ident_b[:],
                        )
                    # sign-evict: raw bf16 -> fp8 {-1,0,1}
                    nc.scalar.activation(
                        out=xqT_vs[ti][:, h * HB:(h + 1) * HB, :],
                        in_=pst[:].rearrange("p (di t) -> p di t", di=HB),
                        func=mybir.ActivationFunctionType.Sign,
                    )

            def w_load(wi):
                wt = wload.tile([P, D], F32)
                nc.sync.dma_start(wt[:], w_d[wi * P:(wi + 1) * P, :])
                wbufs[wi] = {"wt": wt}

            def w_s(wi):
                wt = wbufs[wi]["wt"]
                wcol = slice(wi, wi + 1)
                # junk = -wt/D, accum -> negmean (Pool can't accum;
                # alternate ACT/DVE to split the load)
                junkS = junk_pool.tile([P, D], FP16, tag="junkS")
                if wi % 2 == 0:
                    nc.scalar.activation(
                        out=junkS[:], in_=wt[:],
                        func=mybir.ActivationFunctionType.Copy,
                        scale=-1.0 / D,
                        accum_out=negmean_all[:, wcol],
                    )
                else:
                    nc.vector.tensor_scalar(
                        out=junkS[:], in0=wt[:], scalar1=-1.0 / D, scalar2=0.0,
                        op0=mybir.AluOpType.mult, op1=mybir.AluOpType.add,
                        accum_out=negmean_all[:, wcol],
                    )

            def w_mid(wi):
                wt = wbufs[wi].pop("wt")
                wcol = slice(wi, wi + 1)
                # ACT: aWc = |wt - mean| (f32, exact), accum -> T
                aWc = wtmp.tile([P, D], F32, tag="aWc")
                nc.scalar.activation(
                    out=aWc[:], in_=wt[:],
                    func=mybir.ActivationFunctionType.Abs,
                    bias=negmean_all[:, wcol],
                    accum_out=T_all[:, wcol],
                )
                # DVE tiny: thr = T * delta/D, negthr = -thr
                nc.vector.tensor_scalar_mul(
                    thr_all[:, wcol], T_all[:, wcol], DELTA_W / D)
                nc.vector.tensor_scalar_mul(
                    negthr_all[:, wcol], T_all[:, wcol], -DELTA_W / D)
                # ACT: sgn = Sign(wt - mean) (bf16, exact)
                sgn = wtmp.tile([P, D], BF16, tag="sgn")
                nc.scalar.activation(
                    out=sgn[:], in_=wt[:],
                    func=mybir.ActivationFunctionType.Sign,
                    bias=negmean_all[:, wcol],
                )
                wbufs[wi]["aWc"] = aWc
                wbufs[wi]["sgn"] = sgn

            def w_s01(wi):
                st = wbufs[wi]
                aWc = st["aWc"]
                wcol = slice(wi, wi + 1)
                # DVE: s01 = (aWc >= thr) (bf16), accum -> den
                s01 = wtmp.tile([P, D], BF16, tag="s01")
                nc.vector.tensor_scalar(
                    out=s01[:], in0=aWc[:], scalar1=thr_all[:, wcol],
                    scalar2=0.0, op0=mybir.AluOpType.is_ge,
                    op1=mybir.AluOpType.add,
                    accum_out=den_all[:, wcol],
                )
                # ACT: junk = relu(aWc - thr), accum -> R (num = R + thr*den)
                junkN = junk_pool.tile([P, D], BF16, tag="junkN")
                nc.scalar.activation(
                    out=junkN[:], in_=aWc[:],
                    func=mybir.ActivationFunctionType.Relu,
                    bias=negthr_all[:, wcol],
                    accum_out=R_all[:, wcol],
                )
                st["s01"] = s01

            def w_code(wi):
                st = wbufs.pop(wi)
                sgn, s01 = st["sgn"], st["s01"]
                # DVE: code = sgn * s01 (bf16; exactly -1/0/+1)
                code = wtmp.tile([P, D], BF16, tag="code")
                nc.vector.tensor_mul(code[:], sgn[:], s01[:])
                # PE transposes + DVE evict (bf16 -> fp8 {-1,0,1})
                ob, j_o = divmod(wi, 4)
                for h in range(DT // HB):
                    pst = ps_trw.tile([P, HB * P], BF16, tag="pstw")
                    for j in range(HB):
                        di = h * HB + j
                        nc.tensor.transpose(
                            pst[:, j * P:(j + 1) * P],
                            code[:, di * P:(di + 1) * P],
                            ident_b[:],
                        )
                    nc.vector.tensor_copy(
                        out=codeT_vs[ob][:, h * HB:(h + 1) * HB,
                                         j_o * P:(j_o + 1) * P],
                        in_=pst[:].rearrange("p (di o) -> p di o", di=HB),
                    )

            def alpha_stage(ob):
                cols = slice(4 * ob, 4 * ob + 4)
                # num = R + thr*den; alpha = num / max(den, 1)
                nc.vector.tensor_mul(
                    num_all[:, cols], thr_all[:, cols], den_all[:, cols])
                nc.vector.tensor_add(
                    num_all[:, cols], num_all[:, cols], R_all[:, cols])
                nc.vector.tensor_scalar_max(
                    denc_all[:, cols], den_all[:, cols], 1.0)
                nc.vector.reciprocal(rden_all[:, cols], denc_all[:, cols])
                nc.vector.tensor_mul(
                    alpha_all[:, cols], num_all[:, cols], rden_all[:, cols])
                nc.sync.dma_start(
                    alpha_dram[ob * OBW:(ob + 1) * OBW].rearrange(
                        "(w p) -> p w", p=P)[:, :],
                    alpha_all[:, cols])
                nc.sync.dma_start(
                    alphaB_tiles[ob][:],
                    alpha_dram[ob * OBW:(ob + 1) * OBW].unsqueeze(
                        0).to_broadcast((P, OBW)))

            def mm_stage(ti, ob):
                ps = ps_mm.tile([P, OBW], F32, tag="ps", name=f"ps{ti}_{ob}")
                osl = slice(ob * OBW, (ob + 1) * OBW)
                for dp in range(DP):
                    nc.tensor.matmul(
                        ps[:],
                        xqT_vs[ti][:, 2 * dp:2 * dp + 2, :],
                        codeT_vs[ob][:, 2 * dp:2 * dp + 2, :],
                        start=(dp == 0), stop=(dp == DP - 1),
                        perf_mode=mybir.MatmulPerfMode.DoubleRow,
                    )
                ysb = yout.tile([P, OBW], BF16, tag="ysb")
                # DVE: y = (ps * 1) * alphaB  (GPSIMD cannot read PSUM)
                nc.vector.scalar_tensor_tensor(
                    out=ysb[:], in0=ps[:], scalar=1.0, in1=alphaB_tiles[ob][:],
                    op0=mybir.AluOpType.mult, op1=mybir.AluOpType.mult,
                )
                if with_bias:
                    nc.vector.tensor_add(ysb[:], ysb[:], biasB[:, osl])
                nc.sync.dma_start(
                    y_d[ti * P:(ti + 1) * P, osl], ysb[:])

            # Software-pipelined schedule with a 2-step skew: every sub-stage
            # consumes data produced at least one step earlier, so no engine
            # waits on an intra-step dependency chain.
            # 3-deep skew: load/S -> mid(abs,sgn) -> s01/relu -> code+transpose.
            # Every PE and eviction input is at least one full step old.
            SK = 3
            x_load(0)
            x_load(1)
            w_load(0)
            for k in range(TT + SK):
                # w_code first: its inputs (sgn, s01) are >=1 step old, so the
                # code-mul heads the DVE queue and the PE's W-transposes run
                # without waiting on this step's s01/eviction work.
                if SK <= k:
                    w_code(k - SK)
                if k + 2 < TT:
                    x_load(k + 2)
                if k + 1 < TT:
                    w_load(k + 1)
                if k < TT:
                    x_stage(k)
                # mm groups directly after the x evictions: their y-evictions
                # land ahead of the W-chain in the DVE queue, so PSUM banks
                # recycle without waiting on this step's s01/S work.
                for ti in range(TT):
                    for ob in range(OB):
                        if max(ti, 4 * ob + 3 + SK) == k:
                            mm_stage(ti, ob)
                if k < TT:
                    w_s(k)
                if 1 <= k < TT + 1:
                    w_mid(k - 1)
                if 2 <= k < TT + 2:
                    i = k - 2
                    w_s01(i)
                    if i % 4 == 3:
                        alpha_stage(i // 4)

    nc.compile()
    return nc


def _get_nc(with_bias: bool):
    key = with_bias
    if key not in _CACHE:
        _CACHE[key] = _build(with_bias)
    return _CACHE[key]


def kernel(x: np.ndarray, W: np.ndarray, b: np.ndarray) -> np.ndarray:
    x = np.asarray(x, dtype=np.float32)
    W = np.ascontiguousarray(W, dtype=np.float32)
    b = np.asarray(b, dtype=np.float32)
    with_bias = bool(np.any(b))

    nc = _get_nc(with_bias)

    xf = np.ascontiguousarray(x.reshape(B * S, D))
    in_maps = []
    for c in range(N_CORES):
        m = {"x": np.ascontiguousarray(xf[c * T:(c + 1) * T]), "W": W}
        if with_bias:
            m["b"] = b
        in_maps.append(m)

    res = run_bass_kernel_spmd(nc, in_maps, core_ids=list(range(N_CORES)))
    y = np.concatenate(
        [np.asarray(res.results[c]["y"], dtype=np.float32)
         for c in range(N_CORES)], axis=0)
    return np.ascontiguousarray(y.reshape(B, S, O))


if __name__ == "__main__":
    rng = np.random.default_rng(0)
    x = rng.standard_normal((B, S, D), dtype=np.float32)
    W = rng.standard_normal((O, D), dtype=np.float32) * 0.03
    b = np.zeros((O,), dtype=np.float32)
    y = kernel(x, W, b)
    print("kernel ran, y shape", y.shape, "mean|y|", np.abs(y).mean())
